# revision 47
# baseline (speedup 1.0000x reference)
"""Bass/Trainium2 kernel for nn_DecorrelationGradient.

Reference computation (KAPPA = 0.5):
    out = (1-k)*(gram - diag_ms) + k*(diag_ms - 1)
        = 0.5 * (X^T X / N) - 0.5          (diag terms cancel algebraically)

with X = x.reshape(N, d), N = 8*2048 = 16384, d = 768.

Strategy (data-parallel over the sample axis, 8 cores):
  - core c gets x[c] : [2048, 768] f32
  - per-core pipeline: HWDGE f32 loads (both rings) -> DVE cast to bf16 ->
    PE computes the upper-triangle blocks of the partial Gram P_c = x_c^T x_c
    (bf16 matmuls, fp32 PSUM accumulation over 16 k-tiles; k-outer/i-inner
    so each x tile is consumed right after its DMA+cast lands; the load
    stream runs at the per-core HBM limit and paces the kernel)
  - fused scale+bias on the PSUM->SBUF copy:  t = P_c * (0.5/N) - 0.5/8
  - each core outputs its scaled partial packed triangle [128, 2688] f32;
    the gather step sums the 8 partials (the affine above makes the sum
    equal 0.5*G/N - 0.5) and unpacks the symmetric matrix by indexing.
    (A device-side ReduceScatter variant is kept behind DEVICE_REDUCE; it
    measures ~2.5x slower here because the collective serializes behind a
    cross-core launch-skew barrier plus ncfw trigger latency.)
"""

import numpy as np

import concourse.bacc as bacc
import concourse.bass as bass  # noqa: F401
import concourse.tile as tile
from concourse import mybir
from concourse.bass_utils import run_bass_kernel_spmd

P = 128
D = 768
NSHARD = 2048          # samples per core
KT = NSHARD // P       # 16 k-tiles
NB = D // P            # 6 row/col blocks
NCORES = 8
NTOT = 8 * 2048
SCALE = 0.5 / NTOT     # 2**-15, exact
BIAS = -0.5 / NCORES   # -0.0625, exact; RS adds 8 copies -> -0.5

# packed upper-triangle blocks (i, j) with j >= i, row-major in i
TRI_BLOCKS = [(i, j) for i in range(NB) for j in range(i, NB)]
NTRI = len(TRI_BLOCKS)          # 21
TRI_W = NTRI * P                # 2688 packed columns
ROWS_PER_CORE = P // NCORES     # 16 partition rows of the packed triangle

# packed column range of row-block i
OFFS = []
_o = 0
for _i in range(NB):
    OFFS.append((_o, _o + (NB - _i) * P))
    _o = OFFS[-1][1]


def _copy_out(nc, tri, pss, i):
    """Scaled+biased PSUM->SBUF copy of row-block i, split between the ACT
    and DVE engines so the two halves run in parallel."""
    o0, o1 = OFFS[i]
    w = o1 - o0
    h = (w // 2 + P - 1) // P * P  # split at a 128-col boundary
    nc.scalar.activation(
        out=tri[:, o0 : o0 + h],
        in_=pss[i][:, 0:h],
        func=mybir.ActivationFunctionType.Copy,
        scale=SCALE,
        bias=BIAS,
    )
    if w > h:
        nc.vector.tensor_scalar(
            out=tri[:, o0 + h : o1],
            in0=pss[i][:, h:w],
            scalar1=SCALE,
            scalar2=BIAS,
            op0=mybir.AluOpType.mult,
            op1=mybir.AluOpType.add,
        )


def _split_free(width):
    """Split a moving free-dim into chunks <= 512 (one PSUM bank of fp32)."""
    out = []
    s = 0
    while s < width:
        w = min(512, width - s)
        out.append((s, s + w))
        s += w
    return out


# if True, ReduceScatter on device and each core outputs a [16, 2688] slice;
# if False, each core outputs its full scaled partial triangle [128, 2688]
# and the gather step sums the 8 partials (device collectives pay ~40-60us
# of cross-core launch-skew barrier + ncfw latency here, so off by default)
DEVICE_REDUCE = False


def _build():
    nc = bacc.Bacc(num_devices=NCORES)

    x_sh = nc.dram_tensor(
        "x_shard", [NSHARD, D], mybir.dt.float32, kind="ExternalInput"
    )
    out_shape = [ROWS_PER_CORE, TRI_W] if DEVICE_REDUCE else [P, TRI_W]
    out_sh = nc.dram_tensor(
        "out_shard", out_shape, mybir.dt.float32, kind="ExternalOutput"
    )

    f32 = mybir.dt.float32
    bf16 = mybir.dt.bfloat16

    with tile.TileContext(nc) as tc:
        with (
            tc.tile_pool(name="xp", bufs=KT) as xpool,
            tc.tile_pool(name="bp", bufs=KT) as bpool,
            tc.tile_pool(name="ps", bufs=1, space="PSUM") as pspool,
            tc.tile_pool(name="acc", bufs=1) as accpool,
            tc.tile_pool(name="dram", bufs=1, space="DRAM") as dpool,
        ):
            # HAM warmup tile: first thing in program order so the junk
            # matmuls fill the PE pipe while the first x tiles stream in
            warm = bpool.tile([P, 512], bf16, tag="warm", name="warm")
            nc.gpsimd.memset(warm[:], 0.0)

            # pipeline per k-tile: HWDGE f32 DMA -> DVE cast to bf16 -> PE.
            # loads alternate between the two physical HWDGE rings (SP / ACT)
            xt = []
            for k in range(KT):
                stage = xpool.tile([P, D], f32, tag="xs", name=f"xs{k}")
                dma_eng = nc.sync if k % 2 == 0 else nc.scalar
                dma_eng.dma_start(out=stage[:], in_=x_sh[k * P : (k + 1) * P, :])
                xtile = bpool.tile([P, D], bf16, tag="xb", name=f"xb{k}")
                nc.vector.tensor_copy(out=xtile[:], in_=stage[:])
                xt.append(xtile)

            tri = accpool.tile([P, TRI_W], f32)  # packed scaled triangle
            if DEVICE_REDUCE:
                g_in = dpool.tile([P, TRI_W], f32, name="g_in")
                g_out = dpool.tile([ROWS_PER_CORE, TRI_W], f32, name="g_out")
            dma_dst = g_in if DEVICE_REDUCE else out_sh

            # psum accumulators, one per row-block; exactly 8 PSUM banks.
            # row-block i covers G[i-block, j-blocks j>=i] = cols 128*i..768
            pss = []
            for i in range(NB):
                pss.append(
                    pspool.tile([P, D - P * i], f32, tag=f"ps{i}", name=f"ps{i}")
                )

            # HAM warmup: junk matmuls on the zeroed tile keep the PE busy
            # until the first real tile lands, so the activity window that
            # un-throttles the PE clock starts earlier. Junk goes to
            # pss[0]; the real k=0 matmul has start=True which resets it.
            for w in range(5):
                nc.tensor.matmul(
                    pss[0][:, 0:512],
                    lhsT=warm[:, 0:P],
                    rhs=warm[:],
                    start=True,
                    stop=True,
                )

            # per-k matmul chunk list, ordered so consecutive matmuls use
            # different stationary weights (the 2nd chunk of i=0/i=1 is
            # deferred) - lets the next LDWEIGHTS overlap the running matmul
            chunks = []  # (i, s0, s1, last_of_i)
            deferred = []
            for i in range(NB):
                sp = _split_free(D - P * i)
                chunks.append((i, sp[0][0], sp[0][1], len(sp) == 1))
                for s0, s1 in sp[1:]:
                    deferred.append((i, s0, s1, True))
            chunks[2:2] = deferred  # order: i0a, i1a, i0b, i1b, i2..i5

            # k-outer / i-inner: each x tile is fully consumed right after
            # its DMA+cast lands, so PE overlaps the load stream
            for k in range(KT):
                for i, s0, s1, last_of_i in chunks:
                    c0 = P * i
                    nc.tensor.matmul(
                        pss[i][:, s0:s1],
                        lhsT=xt[k][:, c0 : c0 + P],
                        rhs=xt[k][:, c0 + s0 : c0 + s1],
                        start=(k == 0),
                        stop=(k == KT - 1),
                    )
                    if k == KT - 1 and last_of_i:
                        # last k-tile: copy out each finished row-block while
                        # the PE works on the remaining blocks, and stream the
                        # packed slice to DRAM right away (alternating rings)
                        _copy_out(nc, tri, pss, i)
                        o0, o1 = OFFS[i]
                        dma_eng = nc.sync if i % 2 == 0 else nc.scalar
                        dma_eng.dma_start(
                            out=dma_dst[:, o0:o1], in_=tri[:, o0:o1]
                        )

            if DEVICE_REDUCE:
                nc.gpsimd.collective_compute(
                    "ReduceScatter",
                    mybir.AluOpType.add,
                    replica_groups=[list(range(NCORES))],
                    ins=[g_in.opt()],
                    outs=[g_out.opt()],
                )
                nc.sync.dma_start(out=out_sh[:, :], in_=g_out[:])

    nc.finalize()  # Bacc: run reg-alloc + wait-legalization passes
    return nc


_NC_CACHE = None

# test-harness hooks (harness calls kernel() only; these stay defaults there)
RUN_KWARGS = {}
LAST_RESULTS = None


def _get_nc():
    global _NC_CACHE
    if _NC_CACHE is None:
        _NC_CACHE = _build()
    return _NC_CACHE


def kernel(x: np.ndarray) -> np.ndarray:
    global LAST_RESULTS
    x = np.ascontiguousarray(np.asarray(x, dtype=np.float32))
    assert x.shape == (NCORES, NSHARD, D)

    nc = _get_nc()
    in_maps = [{"x_shard": x[c]} for c in range(NCORES)]
    res = run_bass_kernel_spmd(
        nc, in_maps, core_ids=list(range(NCORES)), **RUN_KWARGS
    )
    LAST_RESULTS = res

    # gather/unshard the packed triangle, then unpack the symmetric matrix
    if DEVICE_REDUCE:
        # concatenate the per-core partition-row slices of the reduced triangle
        packed = np.concatenate(
            [res.results[c]["out_shard"] for c in range(NCORES)], axis=0
        )  # [128, 2688]
    else:
        # sum the per-core scaled partial triangles
        packed = res.results[0]["out_shard"].copy()
        for c in range(1, NCORES):
            packed += res.results[c]["out_shard"]
    packed = packed.reshape(P, NTRI, P).transpose(1, 0, 2)  # [21, 128, 128]

    out = np.empty((D, D), dtype=np.float32)
    for b, (i, j) in enumerate(TRI_BLOCKS):
        blk = packed[b]
        out[P * i : P * (i + 1), P * j : P * (j + 1)] = blk
        if j != i:
            out[P * j : P * (j + 1), P * i : P * (i + 1)] = blk.T
    return out
